# revision 9
# baseline (speedup 1.0000x reference)
"""AttentionBlock (GroupNorm + single-head self-attention + residual) on 8 trn2 cores.

Sharding: core = (batch b = core//2, token-half h = core%2).  Each core gets the
full (128, 4096) channel-major image for its batch (needed for groupnorm stats
and full K/V), computes Q/attention only for its 2048-token half, and writes a
(128, 2048) output slab.  The host rolls the token axis per-core so the q-half
is always columns [0:2048] -> one SPMD program for all 8 cores, no collectives.

Compute layout per core (c = channels on partitions):
  groupnorm stats via bn_stats + two tiny group-mask matmuls (cross-partition)
  hn = alpha*x + beta                        (DVE, one pass)
  q (c,2048), k (c,4096) via f32r matmuls; vT (tok,c) tiles via f32r matmuls
  per q-block (512) x k-triple (3x128):
      scoresT = k_tile^T q_blk  (PE, f32r, PSUM)
      attnT   = exp(scoresT)    (ACT, PSUM->SBUF, 1536-wide instr)
      out    += vT_tile^T attnT (PE, f32r, PSUM accumulate)
      den    += 1^T attnT       (PE ones-matmul / DVE adds, split for balance)
  normalize by 1/den (broadcast via tiny PE matmul), proj, +x residual.

All matmuls are bitcast to float32r: full fp32 data, 1 cycle/row at N>=256
(plain float32 matmul streams at 1/4 rate on trn2).
"""

import numpy as np

C = 128        # channels
N = 4096       # tokens per batch (64*64)
NQ = 2048      # q tokens per core
B = 4
NCORES = 8
GROUPS = 8
EPS = 1e-5
QB = 512       # q block (one PSUM bank of fp32)
NQB = NQ // QB # 4
KT = 128       # k tile (partition dim)
NKT = N // KT  # 32
KT_GROUPS = [3] * 10 + [2]   # k-tile triples (exp instr amortization)

# fraction of k-tiles whose softmax-denominator accumulation runs on DVE
# instead of PE (engine load balancing). kt % DEN_DVE_MOD != 0 -> PE.
DEN_DVE = 2    # every 2nd k-tile's den on DVE

_CACHE = {}


def _build_nc(repeat=1):
    from contextlib import ExitStack

    import concourse.bacc as bacc
    import concourse.bass as bass
    import concourse.mybir as mybir
    import concourse.tile as tile
    from concourse.mybir import ActivationFunctionType as AF
    from concourse.mybir import AluOpType as ALU

    fp32 = mybir.dt.float32
    f32r = mybir.dt.float32r

    nc = bacc.Bacc()

    x_d = nc.dram_tensor("x", [C, N], fp32, kind="ExternalInput")
    wqt_d = nc.dram_tensor("wqt", [C, C], f32r, kind="ExternalInput")
    wkt_d = nc.dram_tensor("wkt", [C, C], f32r, kind="ExternalInput")
    wvt_d = nc.dram_tensor("wvt", [C, C], f32r, kind="ExternalInput")
    wpt_d = nc.dram_tensor("wpt", [C, C], f32r, kind="ExternalInput")
    bq_d = nc.dram_tensor("bq", [C, 1], fp32, kind="ExternalInput")
    bk_d = nc.dram_tensor("bk", [C, 1], fp32, kind="ExternalInput")
    bv_d = nc.dram_tensor("bv", [1, 4 * C], fp32, kind="ExternalInput")  # bv tiled 4x
    bp_d = nc.dram_tensor("bp", [C, 1], fp32, kind="ExternalInput")
    gsc_d = nc.dram_tensor("gscale", [C, 1], fp32, kind="ExternalInput")
    gbi_d = nc.dram_tensor("gbias", [C, 1], fp32, kind="ExternalInput")
    mka_d = nc.dram_tensor("maska", [C, GROUPS], fp32, kind="ExternalInput")
    mkb_d = nc.dram_tensor("maskb", [GROUPS, C], fp32, kind="ExternalInput")
    y_d = nc.dram_tensor("y", [C, NQ], fp32, kind="ExternalOutput")

    with tile.TileContext(nc) as tc, ExitStack() as ctx:
        const = ctx.enter_context(tc.tile_pool(name="const", bufs=1))
        big = ctx.enter_context(tc.tile_pool(name="big", bufs=1))
        small = ctx.enter_context(tc.tile_pool(name="small", bufs=1))

        # ---- load inputs ----
        wqt_sb = const.tile([C, C], f32r)
        nc.sync.dma_start(out=wqt_sb, in_=wqt_d[:, :])
        wkt_sb = const.tile([C, C], f32r)
        nc.sync.dma_start(out=wkt_sb, in_=wkt_d[:, :])
        wvt_sb = const.tile([C, C], f32r)
        nc.sync.dma_start(out=wvt_sb, in_=wvt_d[:, :])
        wpt_sb = const.tile([C, C], f32r)
        nc.sync.dma_start(out=wpt_sb, in_=wpt_d[:, :])
        bq_sb = const.tile([C, 1], fp32)
        nc.sync.dma_start(out=bq_sb, in_=bq_d[:, :])
        bk_sb = const.tile([C, 1], fp32)
        nc.sync.dma_start(out=bk_sb, in_=bk_d[:, :])
        bp_sb = const.tile([C, 1], fp32)
        nc.sync.dma_start(out=bp_sb, in_=bp_d[:, :])
        gsc_sb = const.tile([C, 1], fp32)
        nc.sync.dma_start(out=gsc_sb, in_=gsc_d[:, :])
        gbi_sb = const.tile([C, 1], fp32)
        nc.sync.dma_start(out=gbi_sb, in_=gbi_d[:, :])
        mka_sb = const.tile([C, GROUPS], fp32)
        nc.sync.dma_start(out=mka_sb, in_=mka_d[:, :])
        mkb_sb = const.tile([GROUPS, C], fp32)
        nc.sync.dma_start(out=mkb_sb, in_=mkb_d[:, :])
        # bv broadcast to all 128 partitions, tiled 4x along free dim
        bv_bc = const.tile([C, 4, C], fp32)
        bvap = bv_d[:, :]
        nc.sync.dma_start(
            out=bv_bc,
            in_=bass.AP(tensor=bvap.tensor, offset=0, ap=[[0, C], [1, 4 * C]]),
        )

        ones_col = const.tile([C, 1], fp32)
        nc.vector.memset(ones_col, 1.0)
        ones_col_r = const.tile([C, 1], f32r)
        nc.vector.tensor_copy(ones_col_r, ones_col)
        ones_row = const.tile([1, C], fp32)
        nc.vector.memset(ones_row, 1.0)
        eps_sb = const.tile([C, 1], fp32)
        nc.vector.memset(eps_sb, EPS)

        # ---- repeated body (hw loop for benchmarking; repeat=1 -> plain) ----
        rep_ctx = tc.For_i(0, repeat, 1) if repeat > 1 else None
        if rep_ctx is not None:
            rep_ctx.__enter__()

        x_sb = big.tile([C, N], fp32, tag="x")
        nc.sync.dma_start(out=x_sb, in_=x_d[:, :])

        # ---- groupnorm stats ----
        NCHUNK = N // 512
        with tc.tile_pool(name="stat_ps", bufs=2, space="PSUM") as stat_ps:
            stats = small.tile([C, NCHUNK, 6], fp32)
            for i in range(NCHUNK):
                nc.vector.bn_stats(
                    out=stats[:, i, :], in_=x_sb[:, i * 512 : (i + 1) * 512]
                )
            mv = small.tile([C, 2], fp32)
            nc.vector.bn_aggr(out=mv, in_=stats)

            # S = [m, v + m^2] per channel
            S = small.tile([C, 2], fp32)
            nc.vector.tensor_copy(S[:, 0:1], mv[:, 0:1])
            msq = small.tile([C, 1], fp32)
            nc.vector.tensor_mul(msq, mv[:, 0:1], mv[:, 0:1])
            nc.vector.tensor_add(S[:, 1:2], mv[:, 1:2], msq)

            # group-reduce across partitions via mask matmuls
            g_ps = stat_ps.tile([GROUPS, 2], fp32)
            nc.tensor.matmul(g_ps, mka_sb, S, start=True, stop=True)
            g_sb = small.tile([GROUPS, 2], fp32)
            nc.vector.tensor_copy(g_sb, g_ps)
            g2_ps = stat_ps.tile([C, 2], fp32)
            nc.tensor.matmul(g2_ps, mkb_sb, g_sb, start=True, stop=True)

            gsz = C // GROUPS  # channels per group
            mean_g = small.tile([C, 1], fp32)
            nc.vector.tensor_scalar_mul(mean_g, g2_ps[:, 0:1], 1.0 / gsz)
            e2_g = small.tile([C, 1], fp32)
            nc.vector.tensor_scalar_mul(e2_g, g2_ps[:, 1:2], 1.0 / gsz)
            var_g = small.tile([C, 1], fp32)
            nc.vector.tensor_mul(var_g, mean_g, mean_g)
            nc.vector.tensor_tensor(out=var_g, in0=e2_g, in1=var_g, op=ALU.subtract)
            std_g = small.tile([C, 1], fp32)
            nc.scalar.activation(std_g, var_g, AF.Sqrt, bias=eps_sb, scale=1.0)
            rstd_g = small.tile([C, 1], fp32)
            nc.vector.reciprocal(rstd_g, std_g)
            alpha = small.tile([C, 1], fp32)
            nc.vector.tensor_mul(alpha, rstd_g, gsc_sb)
            beta = small.tile([C, 1], fp32)
            nc.vector.tensor_mul(beta, mean_g, alpha)
            nc.vector.tensor_tensor(out=beta, in0=gbi_sb, in1=beta, op=ALU.subtract)

        # ---- hn = alpha*x + beta (one DVE pass) ----
        hn = big.tile([C, N], f32r)
        nc.vector.tensor_scalar(
            out=hn, in0=x_sb, scalar1=alpha, scalar2=beta, op0=ALU.mult, op1=ALU.add
        )

        # ---- q (half), k (full), vT (full) ----
        q_sb = big.tile([C, NQB, QB], f32r)
        k_sb = big.tile([C, N], f32r)
        vT_sb = big.tile([KT, NKT, C], f32r)

        with (
            tc.tile_pool(name="qk_ps", bufs=3, space="PSUM") as qk_ps,
            tc.tile_pool(name="v_ps", bufs=2, space="PSUM") as v_ps,
        ):
            for j in range(NQB):
                ps = qk_ps.tile([C, 512], fp32, tag="qk")
                nc.tensor.matmul(
                    ps, wqt_sb, hn[:, j * 512 : (j + 1) * 512],
                    start=True, stop=True,
                )
                nc.vector.tensor_scalar_add(q_sb[:, j, :], ps, bq_sb)
            for j in range(N // 512):
                ps = qk_ps.tile([C, 512], fp32, tag="qk")
                nc.tensor.matmul(
                    ps, wkt_sb, hn[:, j * 512 : (j + 1) * 512],
                    start=True, stop=True,
                )
                nc.scalar.activation(
                    k_sb[:, j * 512 : (j + 1) * 512], ps, AF.Identity,
                    bias=bk_sb, scale=1.0,
                )
            for g in range(NKT // 4):
                ps = v_ps.tile([KT, 4, C], fp32, tag="v")
                for t in range(4):
                    kt = g * 4 + t
                    nc.tensor.matmul(
                        ps[:, t, :],
                        hn[:, kt * KT : (kt + 1) * KT],
                        wvt_sb,
                        start=True,
                        stop=True,
                    )
                # vT = psum + bv (broadcast), one DVE pass per group of 4
                nc.vector.tensor_tensor(
                    out=vT_sb[:, g * 4 : (g + 1) * 4, :], in0=ps, in1=bv_bc,
                    op=ALU.add,
                )

        # ---- attention main loop ----
        aout_sb = big.tile([C, NQB, QB], f32r)
        with (
            tc.tile_pool(name="s_ps", bufs=2, space="PSUM") as spool,
            tc.tile_pool(name="o_ps", bufs=1, space="PSUM") as opool,
            tc.tile_pool(name="d_ps", bufs=1, space="PSUM") as dpool,
            tc.tile_pool(name="attn", bufs=2) as apool,
        ):
            for qb in range(NQB):
                out_ps = opool.tile([C, QB], fp32, tag="out")
                den_ps = dpool.tile([1, QB], fp32, tag="den")
                den_sb = small.tile([KT, QB], fp32, tag="densb", bufs=2)
                qv = q_sb[:, qb, :]
                kt = 0
                dve_den_started = False
                for gi, gsize in enumerate(KT_GROUPS):
                    s_ps = spool.tile([KT, 3, QB], fp32, tag="s")
                    for t in range(gsize):
                        nc.tensor.matmul(
                            s_ps[:, t, :],
                            k_sb[:, (kt + t) * KT : (kt + t + 1) * KT],
                            qv,
                            start=True,
                            stop=True,
                        )
                    at = apool.tile([KT, 3, QB], f32r, tag="at")
                    if gsize == 3:
                        nc.scalar.activation(at, s_ps, AF.Exp)
                    else:
                        nc.scalar.activation(
                            at[:, :gsize, :], s_ps[:, :gsize, :], AF.Exp
                        )
                    for t in range(gsize):
                        k_idx = kt + t
                        nc.tensor.matmul(
                            out_ps,
                            vT_sb[:, k_idx, :],
                            at[:, t, :],
                            start=(k_idx == 0),
                            stop=(k_idx == NKT - 1),
                        )
                        if k_idx % DEN_DVE == 0:
                            # PE path for denominator
                            nc.tensor.matmul(
                                den_ps,
                                ones_col_r,
                                at[:, t, :],
                                start=(k_idx == 0),
                                stop=False,
                            )
                        else:
                            # DVE path for denominator
                            if not dve_den_started:
                                nc.vector.tensor_copy(den_sb, at[:, t, :])
                                dve_den_started = True
                            else:
                                nc.vector.tensor_add(den_sb, den_sb, at[:, t, :])
                    kt += gsize
                # fold the DVE part into den_ps (completes the accumulation group)
                nc.tensor.matmul(den_ps, ones_col, den_sb, start=False, stop=True)

                # normalize: aout = out * (1/den) broadcast along partitions
                rden = small.tile([1, QB], fp32, tag="rden", bufs=2)
                nc.vector.reciprocal(rden, den_ps)
                bc_ps = spool.tile([KT, 3, QB], fp32, tag="s")
                nc.tensor.matmul(
                    bc_ps[:, 0, :], ones_row, rden, start=True, stop=True
                )
                rbc = small.tile([C, QB], fp32, tag="rbc", bufs=2)
                nc.vector.tensor_copy(rbc, bc_ps[:, 0, :])
                nc.vector.tensor_mul(aout_sb[:, qb, :], out_ps, rbc)

                # proj + bias + residual for this q block
                pp = spool.tile([KT, 3, QB], fp32, tag="s")
                nc.tensor.matmul(
                    pp[:, 0, :], wpt_sb, aout_sb[:, qb, :], start=True, stop=True
                )
                y_sb = small.tile([C, QB], fp32, tag="y", bufs=2)
                nc.vector.tensor_tensor(
                    out=y_sb, in0=pp[:, 0, :], in1=x_sb[:, qb * QB : (qb + 1) * QB],
                    op=ALU.add,
                )
                nc.vector.tensor_scalar_add(y_sb, y_sb, bp_sb)
                nc.sync.dma_start(out=y_d[:, qb * QB : (qb + 1) * QB], in_=y_sb)

        if rep_ctx is not None:
            rep_ctx.__exit__(None, None, None)

    nc.compile()
    return nc


def _prep_maps(x):
    x = np.ascontiguousarray(np.asarray(x, dtype=np.float32))
    b, c, h, w = x.shape
    assert (b, c, h * w) == (B, C, N), f"unexpected shape {x.shape}"
    return x.reshape(b, c, h * w)


def _make_in_maps(x, norm_scale, norm_bias, wq, bq, wk, bk, wv, bv, wp, bp):
    xr = _prep_maps(x)
    s = float(C) ** -0.5
    f32 = np.float32

    wqt = np.ascontiguousarray((np.asarray(wq, f32) * s).T)
    wkt = np.ascontiguousarray(np.asarray(wk, f32).T)
    wvt = np.ascontiguousarray(np.asarray(wv, f32).T)
    wpt = np.ascontiguousarray(np.asarray(wp, f32).T)
    bqv = np.ascontiguousarray((np.asarray(bq, f32) * s).reshape(C, 1))
    bkv = np.ascontiguousarray(np.asarray(bk, f32).reshape(C, 1))
    bvv = np.ascontiguousarray(np.tile(np.asarray(bv, f32).reshape(1, C), (1, 4)))
    bpv = np.ascontiguousarray(np.asarray(bp, f32).reshape(C, 1))
    gsc = np.ascontiguousarray(np.asarray(norm_scale, f32).reshape(C, 1))
    gbi = np.ascontiguousarray(np.asarray(norm_bias, f32).reshape(C, 1))
    maska = np.zeros((C, GROUPS), f32)
    maska[np.arange(C), np.arange(C) // (C // GROUPS)] = 1.0
    maskb = np.ascontiguousarray(maska.T)

    in_maps = []
    for core in range(NCORES):
        bi, hi = core // 2, core % 2
        xb = xr[bi]
        if hi:
            xb = np.roll(xb, -NQ, axis=1)
        in_maps.append(
            dict(
                x=np.ascontiguousarray(xb),
                wqt=wqt, wkt=wkt, wvt=wvt, wpt=wpt,
                bq=bqv, bk=bkv, bv=bvv, bp=bpv,
                gscale=gsc, gbias=gbi, maska=maska, maskb=maskb,
            )
        )
    return in_maps


def kernel(x, norm_scale, norm_bias, wq, bq, wk, bk, wv, bv, wp, bp):
    from concourse.bass_utils import run_bass_kernel_spmd

    in_maps = _make_in_maps(
        x, norm_scale, norm_bias, wq, bq, wk, bk, wv, bv, wp, bp
    )

    if "nc" not in _CACHE:
        _CACHE["nc"] = _build_nc()
    res = run_bass_kernel_spmd(
        _CACHE["nc"], in_maps, core_ids=list(range(NCORES)), **_CACHE.get("runkw", {})
    )
    _CACHE["last_result"] = res

    out = np.empty((B, C, N), np.float32)
    for core in range(NCORES):
        bi, hi = core // 2, core % 2
        out[bi, :, hi * NQ : (hi + 1) * NQ] = res.results[core]["y"]
    return out.reshape(B, C, 64, 64)


# revision 11
# speedup vs baseline: 3.1247x; 3.1247x over previous
"""AttentionBlock (GroupNorm + single-head self-attention + residual) on 8 trn2 cores.

Sharding: core = (batch b = core//2, token-half h = core%2).  Each core gets the
full (128, 4096) channel-major image for its batch (needed for groupnorm stats
and full K/V), computes Q/attention only for its 2048-token half, and writes a
(128, 2048) output slab.  The host rolls the token axis per-core so the q-half
is always columns [0:2048] -> one SPMD program for all 8 cores, no collectives.

Compute layout per core (c = channels on partitions):
  groupnorm stats via bn_stats + two tiny group-mask matmuls (cross-partition)
  hn = alpha*x + beta                        (DVE, one pass)
  q (c,2048), k (c,4096) via f32r matmuls; vT (tok,c) tiles via f32r matmuls
  per q-block (512) x k-triple (3x128):
      scoresT = k_tile^T q_blk  (PE, f32r, PSUM)
      attnT   = exp(scoresT)    (ACT, PSUM->SBUF, 1536-wide instr)
      out    += vT_tile^T attnT (PE, f32r, PSUM accumulate)
      den    += 1^T attnT       (PE ones-matmul / DVE adds, split for balance)
  normalize by 1/den (broadcast via tiny PE matmul), proj, +x residual.

All matmuls are bitcast to float32r: full fp32 data, 1 cycle/row at N>=256
(plain float32 matmul streams at 1/4 rate on trn2).
"""

import numpy as np

C = 128        # channels
N = 4096       # tokens per batch (64*64)
NQ = 2048      # q tokens per core
B = 4
NCORES = 8
GROUPS = 8
EPS = 1e-5
QB = 512       # q block (one PSUM bank of fp32)
NQB = NQ // QB # 4
KT = 128       # k tile (partition dim)
NKT = N // KT  # 32
KT_GROUPS = [3] * 10 + [2]   # k-tile triples (exp instr amortization)

# fraction of k-tiles whose softmax-denominator accumulation runs on DVE
# instead of PE (engine load balancing). kt % DEN_DVE_MOD != 0 -> PE.
DEN_DVE = 2    # every 2nd k-tile's den on DVE

_CACHE = {}


def _build_nc(repeat=1):
    from contextlib import ExitStack

    import concourse.bacc as bacc
    import concourse.bass as bass
    import concourse.mybir as mybir
    import concourse.tile as tile
    from concourse.mybir import ActivationFunctionType as AF
    from concourse.mybir import AluOpType as ALU

    fp32 = mybir.dt.float32
    f32r = mybir.dt.float32r

    nc = bacc.Bacc()

    x_d = nc.dram_tensor("x", [C, N], fp32, kind="ExternalInput")
    wqt_d = nc.dram_tensor("wqt", [C, C], f32r, kind="ExternalInput")
    wkt_d = nc.dram_tensor("wkt", [C, C], f32r, kind="ExternalInput")
    wvt_d = nc.dram_tensor("wvt", [C, C], f32r, kind="ExternalInput")
    wpt_d = nc.dram_tensor("wpt", [C, C], f32r, kind="ExternalInput")
    bq_d = nc.dram_tensor("bq", [C, 1], fp32, kind="ExternalInput")
    bk_d = nc.dram_tensor("bk", [C, 1], fp32, kind="ExternalInput")
    bv_d = nc.dram_tensor("bv", [1, 4 * C], fp32, kind="ExternalInput")  # bv tiled 4x
    bp_d = nc.dram_tensor("bp", [C, 1], fp32, kind="ExternalInput")
    gsc_d = nc.dram_tensor("gscale", [C, 1], fp32, kind="ExternalInput")
    gbi_d = nc.dram_tensor("gbias", [C, 1], fp32, kind="ExternalInput")
    mka_d = nc.dram_tensor("maska", [C, GROUPS], fp32, kind="ExternalInput")
    mkb_d = nc.dram_tensor("maskb", [GROUPS, C], fp32, kind="ExternalInput")
    y_d = nc.dram_tensor("y", [C, NQ], fp32, kind="ExternalOutput")

    with tile.TileContext(nc) as tc, ExitStack() as ctx:
        const = ctx.enter_context(tc.tile_pool(name="const", bufs=1))
        big = ctx.enter_context(tc.tile_pool(name="big", bufs=1))
        small = ctx.enter_context(tc.tile_pool(name="small", bufs=1))

        # ---- load inputs ----
        wqt_sb = const.tile([C, C], f32r)
        nc.sync.dma_start(out=wqt_sb, in_=wqt_d[:, :])
        wkt_sb = const.tile([C, C], f32r)
        nc.sync.dma_start(out=wkt_sb, in_=wkt_d[:, :])
        wvt_sb = const.tile([C, C], f32r)
        nc.sync.dma_start(out=wvt_sb, in_=wvt_d[:, :])
        wpt_sb = const.tile([C, C], f32r)
        nc.sync.dma_start(out=wpt_sb, in_=wpt_d[:, :])
        bq_sb = const.tile([C, 1], fp32)
        nc.sync.dma_start(out=bq_sb, in_=bq_d[:, :])
        bk_sb = const.tile([C, 1], fp32)
        nc.sync.dma_start(out=bk_sb, in_=bk_d[:, :])
        bp_sb = const.tile([C, 1], fp32)
        nc.sync.dma_start(out=bp_sb, in_=bp_d[:, :])
        gsc_sb = const.tile([C, 1], fp32)
        nc.sync.dma_start(out=gsc_sb, in_=gsc_d[:, :])
        gbi_sb = const.tile([C, 1], fp32)
        nc.sync.dma_start(out=gbi_sb, in_=gbi_d[:, :])
        mka_sb = const.tile([C, GROUPS], fp32)
        nc.sync.dma_start(out=mka_sb, in_=mka_d[:, :])
        mkb_sb = const.tile([GROUPS, C], fp32)
        nc.sync.dma_start(out=mkb_sb, in_=mkb_d[:, :])
        # bv broadcast to all 128 partitions, tiled 4x along free dim
        bv_bc = const.tile([C, 4, C], fp32)
        bvap = bv_d[:, :]
        nc.sync.dma_start(
            out=bv_bc,
            in_=bass.AP(tensor=bvap.tensor, offset=0, ap=[[0, C], [1, 4 * C]]),
        )

        ones_col = const.tile([C, 1], fp32)
        nc.vector.memset(ones_col, 1.0)
        ones_col_r = const.tile([C, 1], f32r)
        nc.vector.tensor_copy(ones_col_r, ones_col)
        ones_row = const.tile([1, C], fp32)
        nc.vector.memset(ones_row, 1.0)
        eps_sb = const.tile([C, 1], fp32)
        nc.vector.memset(eps_sb, EPS)

        # ---- repeated body (hw loop for benchmarking; repeat=1 -> plain) ----
        rep_ctx = tc.For_i(0, repeat, 1) if repeat > 1 else None
        if rep_ctx is not None:
            rep_ctx.__enter__()

        x_sb = big.tile([C, N], fp32, tag="x")
        nc.sync.dma_start(out=x_sb, in_=x_d[:, :])

        # ---- groupnorm stats ----
        NCHUNK = N // 512
        with tc.tile_pool(name="stat_ps", bufs=2, space="PSUM") as stat_ps:
            stats = small.tile([C, NCHUNK, 6], fp32)
            for i in range(NCHUNK):
                nc.vector.bn_stats(
                    out=stats[:, i, :], in_=x_sb[:, i * 512 : (i + 1) * 512]
                )
            mv = small.tile([C, 2], fp32)
            nc.vector.bn_aggr(out=mv, in_=stats)

            # S = [m, v + m^2] per channel
            S = small.tile([C, 2], fp32)
            nc.vector.tensor_copy(S[:, 0:1], mv[:, 0:1])
            msq = small.tile([C, 1], fp32)
            nc.vector.tensor_mul(msq, mv[:, 0:1], mv[:, 0:1])
            nc.vector.tensor_add(S[:, 1:2], mv[:, 1:2], msq)

            # group-reduce across partitions via mask matmuls
            g_ps = stat_ps.tile([GROUPS, 2], fp32)
            nc.tensor.matmul(g_ps, mka_sb, S, start=True, stop=True)
            g_sb = small.tile([GROUPS, 2], fp32)
            nc.vector.tensor_copy(g_sb, g_ps)
            g2_ps = stat_ps.tile([C, 2], fp32)
            nc.tensor.matmul(g2_ps, mkb_sb, g_sb, start=True, stop=True)

            gsz = C // GROUPS  # channels per group
            mean_g = small.tile([C, 1], fp32)
            nc.vector.tensor_scalar_mul(mean_g, g2_ps[:, 0:1], 1.0 / gsz)
            e2_g = small.tile([C, 1], fp32)
            nc.vector.tensor_scalar_mul(e2_g, g2_ps[:, 1:2], 1.0 / gsz)
            var_g = small.tile([C, 1], fp32)
            nc.vector.tensor_mul(var_g, mean_g, mean_g)
            nc.vector.tensor_tensor(out=var_g, in0=e2_g, in1=var_g, op=ALU.subtract)
            std_g = small.tile([C, 1], fp32)
            nc.scalar.activation(std_g, var_g, AF.Sqrt, bias=eps_sb, scale=1.0)
            rstd_g = small.tile([C, 1], fp32)
            nc.vector.reciprocal(rstd_g, std_g)
            alpha = small.tile([C, 1], fp32)
            nc.vector.tensor_mul(alpha, rstd_g, gsc_sb)
            beta = small.tile([C, 1], fp32)
            nc.vector.tensor_mul(beta, mean_g, alpha)
            nc.vector.tensor_tensor(out=beta, in0=gbi_sb, in1=beta, op=ALU.subtract)

        # ---- hn = alpha*x + beta (one DVE pass) ----
        hn = big.tile([C, N], f32r)
        nc.vector.tensor_scalar(
            out=hn, in0=x_sb, scalar1=alpha, scalar2=beta, op0=ALU.mult, op1=ALU.add
        )

        # ---- q (half), k (full), vT (full) ----
        q_sb = big.tile([C, NQB, QB], f32r)
        k_sb = big.tile([C, N], f32r)
        vT_sb = big.tile([KT, NKT, C], f32r)

        with (
            tc.tile_pool(name="qk_ps", bufs=3, space="PSUM") as qk_ps,
            tc.tile_pool(name="v_ps", bufs=2, space="PSUM") as v_ps,
        ):
            for j in range(NQB):
                ps = qk_ps.tile([C, 512], fp32, tag="qk")
                nc.tensor.matmul(
                    ps, wqt_sb, hn[:, j * 512 : (j + 1) * 512],
                    start=True, stop=True,
                )
                nc.vector.tensor_scalar_add(q_sb[:, j, :], ps, bq_sb)
            for j in range(N // 512):
                ps = qk_ps.tile([C, 512], fp32, tag="qk")
                nc.tensor.matmul(
                    ps, wkt_sb, hn[:, j * 512 : (j + 1) * 512],
                    start=True, stop=True,
                )
                nc.scalar.activation(
                    k_sb[:, j * 512 : (j + 1) * 512], ps, AF.Identity,
                    bias=bk_sb, scale=1.0,
                )
            for g in range(NKT // 4):
                ps = v_ps.tile([KT, 4, C], fp32, tag="v")
                for t in range(4):
                    kt = g * 4 + t
                    nc.tensor.matmul(
                        ps[:, t, :],
                        hn[:, kt * KT : (kt + 1) * KT],
                        wvt_sb,
                        start=True,
                        stop=True,
                    )
                # vT = psum + bv (broadcast), one DVE pass per group of 4
                nc.vector.tensor_tensor(
                    out=vT_sb[:, g * 4 : (g + 1) * 4, :], in0=ps, in1=bv_bc,
                    op=ALU.add,
                )

        # ---- attention main loop ----
        aout_sb = big.tile([C, NQB, QB], f32r)
        with (
            tc.tile_pool(name="s_ps", bufs=2, space="PSUM") as spool,
            tc.tile_pool(name="o_ps", bufs=1, space="PSUM") as opool,
            tc.tile_pool(name="d_ps", bufs=1, space="PSUM") as dpool,
            tc.tile_pool(name="attn", bufs=3) as apool,
        ):
            for qb in range(NQB):
                out_ps = opool.tile([C, QB], fp32, tag="out")
                den_ps = dpool.tile([1, QB], fp32, tag="den")
                den_sb = small.tile([KT, QB], fp32, tag="densb", bufs=2)
                qv = q_sb[:, qb, :]
                kt = 0
                dve_den_started = False
                for gi, gsize in enumerate(KT_GROUPS):
                    s_ps = spool.tile([KT, 3, QB], fp32, tag="s")
                    for t in range(gsize):
                        nc.tensor.matmul(
                            s_ps[:, t, :],
                            k_sb[:, (kt + t) * KT : (kt + t + 1) * KT],
                            qv,
                            start=True,
                            stop=True,
                        )
                    at = apool.tile([KT, 3, QB], f32r, tag="at")
                    nc.scalar.activation(at[:, :gsize, :], s_ps[:, :gsize, :], AF.Exp)
                    for t in range(gsize):
                        k_idx = kt + t
                        nc.tensor.matmul(
                            out_ps,
                            vT_sb[:, k_idx, :],
                            at[:, t, :],
                            start=(k_idx == 0),
                            stop=(k_idx == NKT - 1),
                        )
                        if (k_idx % 8) not in (3, 5, 7):
                            # PE path for denominator
                            nc.tensor.matmul(
                                den_ps,
                                ones_col_r,
                                at[:, t, :],
                                start=(k_idx == 0),
                                stop=False,
                            )
                        else:
                            # DVE path for denominator
                            if not dve_den_started:
                                nc.vector.tensor_copy(den_sb, at[:, t, :])
                                dve_den_started = True
                            else:
                                nc.vector.tensor_add(den_sb, den_sb, at[:, t, :])
                    kt += gsize
                # fold the DVE part into den_ps (completes the accumulation group)
                nc.tensor.matmul(den_ps, ones_col, den_sb, start=False, stop=True)

                # normalize: aout = out * (1/den) broadcast along partitions
                rden = small.tile([1, QB], fp32, tag="rden", bufs=2)
                nc.vector.reciprocal(rden, den_ps)
                bc_ps = spool.tile([KT, 3, QB], fp32, tag="s")
                nc.tensor.matmul(
                    bc_ps[:, 0, :], ones_row, rden, start=True, stop=True
                )
                rbc = small.tile([C, QB], fp32, tag="rbc", bufs=2)
                nc.vector.tensor_copy(rbc, bc_ps[:, 0, :])
                nc.vector.tensor_mul(aout_sb[:, qb, :], out_ps, rbc)

                # proj + bias + residual for this q block
                pp = spool.tile([KT, 3, QB], fp32, tag="s")
                nc.tensor.matmul(
                    pp[:, 0, :], wpt_sb, aout_sb[:, qb, :], start=True, stop=True
                )
                y_sb = small.tile([C, QB], fp32, tag="y", bufs=2)
                nc.vector.tensor_tensor(
                    out=y_sb, in0=pp[:, 0, :], in1=x_sb[:, qb * QB : (qb + 1) * QB],
                    op=ALU.add,
                )
                nc.vector.tensor_scalar_add(y_sb, y_sb, bp_sb)
                nc.sync.dma_start(out=y_d[:, qb * QB : (qb + 1) * QB], in_=y_sb)

        if rep_ctx is not None:
            rep_ctx.__exit__(None, None, None)

    nc.compile()
    return nc


def _prep_maps(x):
    x = np.ascontiguousarray(np.asarray(x, dtype=np.float32))
    b, c, h, w = x.shape
    assert (b, c, h * w) == (B, C, N), f"unexpected shape {x.shape}"
    return x.reshape(b, c, h * w)


def _make_in_maps(x, norm_scale, norm_bias, wq, bq, wk, bk, wv, bv, wp, bp):
    xr = _prep_maps(x)
    s = float(C) ** -0.5
    f32 = np.float32

    wqt = np.ascontiguousarray((np.asarray(wq, f32) * s).T)
    wkt = np.ascontiguousarray(np.asarray(wk, f32).T)
    wvt = np.ascontiguousarray(np.asarray(wv, f32).T)
    wpt = np.ascontiguousarray(np.asarray(wp, f32).T)
    bqv = np.ascontiguousarray((np.asarray(bq, f32) * s).reshape(C, 1))
    bkv = np.ascontiguousarray(np.asarray(bk, f32).reshape(C, 1))
    bvv = np.ascontiguousarray(np.tile(np.asarray(bv, f32).reshape(1, C), (1, 4)))
    bpv = np.ascontiguousarray(np.asarray(bp, f32).reshape(C, 1))
    gsc = np.ascontiguousarray(np.asarray(norm_scale, f32).reshape(C, 1))
    gbi = np.ascontiguousarray(np.asarray(norm_bias, f32).reshape(C, 1))
    maska = np.zeros((C, GROUPS), f32)
    maska[np.arange(C), np.arange(C) // (C // GROUPS)] = 1.0
    maskb = np.ascontiguousarray(maska.T)

    in_maps = []
    for core in range(NCORES):
        bi, hi = core // 2, core % 2
        xb = xr[bi]
        if hi:
            xb = np.roll(xb, -NQ, axis=1)
        in_maps.append(
            dict(
                x=np.ascontiguousarray(xb),
                wqt=wqt, wkt=wkt, wvt=wvt, wpt=wpt,
                bq=bqv, bk=bkv, bv=bvv, bp=bpv,
                gscale=gsc, gbias=gbi, maska=maska, maskb=maskb,
            )
        )
    return in_maps


def kernel(x, norm_scale, norm_bias, wq, bq, wk, bk, wv, bv, wp, bp):
    from concourse.bass_utils import run_bass_kernel_spmd

    in_maps = _make_in_maps(
        x, norm_scale, norm_bias, wq, bq, wk, bk, wv, bv, wp, bp
    )

    if "nc" not in _CACHE:
        _CACHE["nc"] = _build_nc()
    res = run_bass_kernel_spmd(
        _CACHE["nc"], in_maps, core_ids=list(range(NCORES)), **_CACHE.get("runkw", {})
    )
    _CACHE["last_result"] = res

    out = np.empty((B, C, N), np.float32)
    for core in range(NCORES):
        bi, hi = core // 2, core % 2
        out[bi, :, hi * NQ : (hi + 1) * NQ] = res.results[core]["y"]
    return out.reshape(B, C, 64, 64)
